# revision 2
# baseline (speedup 1.0000x reference)
"""Trainium2 Bass kernel for a 2-layer LSTM (B=64, T=256, N=M=1024).

Strategy (data-parallel, per sharding hint):
 - Shard batch B=64 across 8 cores (8 per core). Weights replicated.
 - All layout transforms (transposes, bf16 casts) happen host-side; the
   device does only FLOPs.
 - Per core, per layer:
     Phase bulk: GxT = WxT_l @ inT for all timesteps at once (near-peak
       bf16 matmul), + bias, stored bf16 in DRAM scratch, "transposed"
       layout: hidden/gate dim on partitions, (t, b) on free dim.
     Phase scan: 256 sequential steps. Per step, gates_h = Wh_l @ h_t^T
       via 256 (128x128) weight-stationary matmuls (weight-load bound),
       elementwise (sigmoid/tanh/mul/add) in transposed layout so the
       128-lane engines are fully used. i/f/g/o land in separate PSUM
       banks so the c/h elementwise chain overlaps the tail matmuls.
 - Scan runs under tc.For_i with dynamic DRAM offsets.
"""

import numpy as np
import ml_dtypes

import concourse.bass as bass
import concourse.bacc as bacc
import concourse.mybir as mybir
import concourse.tile as tile
from concourse.bass import ds
from concourse.bass_utils import run_bass_kernel_spmd

BF16 = ml_dtypes.bfloat16

N_CORES = 8
B, T_FULL, N_IN = 64, 256, 1024
M, L = 1024, 2
BL = B // N_CORES          # 8 batch per core
P = 128
KK = M // P                # 8 contraction tiles
RT = (4 * M) // P          # 32 gate row tiles
KB = KK * BL               # 64 = free width of state tiles
G4 = 4 * M                 # 4096


def _build(T, unroll=2):
    """Build + compile the per-core program. Returns nc."""
    FD = min(512, T * BL)          # bulk free dim per matmul
    NCH = (T * BL) // FD           # bulk chunks
    TC = FD // BL                  # timesteps per bulk chunk
    f32 = mybir.dt.float32
    bf16 = mybir.dt.bfloat16
    AF = mybir.ActivationFunctionType
    PE = mybir.EngineType.PE

    nc = bacc.Bacc("TRN2", target_bir_lowering=False, debug=False,
                   num_devices=N_CORES)

    xt = nc.dram_tensor("xt", [T, P, KB], bf16, kind="ExternalInput")
    # wt[l, 0] = Wx_l^T, wt[l, 1] = Wh_l^T, rows = kk*P + p (contraction), cols = gate
    wt = nc.dram_tensor("wt", [L, 2, KK * P, G4], bf16, kind="ExternalInput")
    bt = nc.dram_tensor("bt", [P, L * RT], f32, kind="ExternalInput")
    h0 = nc.dram_tensor("h0", [L, P, KB], f32, kind="ExternalInput")
    c0 = nc.dram_tensor("c0", [L, P, KB], f32, kind="ExternalInput")
    outst = nc.dram_tensor("outst", [T, P, KB], f32, kind="ExternalOutput")
    hnt = nc.dram_tensor("hnt", [L, P, KB], f32, kind="ExternalOutput")
    cnt = nc.dram_tensor("cnt", [L, P, KB], f32, kind="ExternalOutput")

    with tile.TileContext(nc) as tc:
        with (
            tc.tile_pool(name="w", bufs=1) as wpool,
            tc.tile_pool(name="sb", bufs=3) as sb,
            tc.tile_pool(name="gxp", bufs=3) as gxp,
            tc.tile_pool(name="elt", bufs=2) as elt,
            tc.tile_pool(name="st", bufs=1) as st,
            tc.tile_pool(name="ps", bufs=2, space="PSUM") as psp,
            tc.tile_pool(name="dram", bufs=1, space="DRAM") as dram,
        ):
            bt_sb = st.tile([P, L * RT], f32, tag="bt")
            nc.sync.dma_start(bt_sb[:], bt.ap())
            h0t = dram.tile([T, P, KB], bf16, tag="h0t")

            for l in range(L):
                # ---------------- bulk: GxT = WxT_l @ in^T + b ----------------
                src3d = xt.ap() if l == 0 else h0t[:]
                w_sb = wpool.tile([P, KK * G4], bf16, tag="w")
                nc.sync.dma_start(
                    w_sb[:].rearrange("p (kk g) -> p kk g", kk=KK),
                    wt.ap()[l, 0].rearrange("(kk p) g -> p kk g", p=P),
                )
                gxt = dram.tile([RT, P, T * BL], bf16, tag="gxt")
                for c in range(NCH):
                    rhs = sb.tile([P, KK, FD], bf16, tag="rhs")
                    nc.sync.dma_start(
                        rhs[:].rearrange("p kk (t b) -> p kk t b", b=BL),
                        src3d[c * TC:(c + 1) * TC, :, :]
                        .rearrange("t p (kk b) -> p kk t b", b=BL),
                    )
                    for rt in range(RT):
                        ps = psp.tile([P, FD], f32, tag="bulkps")
                        for kk in range(KK):
                            nc.tensor.matmul(
                                ps[:],
                                w_sb[:, kk * G4 + rt * P: kk * G4 + (rt + 1) * P],
                                rhs[:, kk, :],
                                start=(kk == 0), stop=(kk == KK - 1),
                            )
                        ev = sb.tile([P, FD], bf16, tag="ev")
                        nc.vector.tensor_scalar_add(
                            ev[:], ps[:], bt_sb[:, l * RT + rt: l * RT + rt + 1])
                        nc.sync.dma_start(gxt[rt, :, c * FD:(c + 1) * FD], ev[:])

                # ---------------- scan over T steps ----------------
                w_sb2 = wpool.tile([P, KK * G4], bf16, tag="w")
                nc.sync.dma_start(
                    w_sb2[:].rearrange("p (kk g) -> p kk g", kk=KK),
                    wt.ap()[l, 1].rearrange("(kk p) g -> p kk g", p=P),
                )
                h_bf = [st.tile([P, KB], bf16, tag=f"hbf{j}l{l}", name=f"hbf{j}l{l}") for j in range(2)]
                hf = [st.tile([P, KB], f32, tag=f"hf{j}l{l}", name=f"hf{j}l{l}") for j in range(2)]
                cf = st.tile([P, KB], f32, tag=f"cfl{l}")
                nc.sync.dma_start(hf[0][:], h0.ap()[l])
                nc.sync.dma_start(cf[:], c0.ap()[l])
                nc.vector.tensor_copy(h_bf[0][:], hf[0][:])

                with tc.For_i(0, T, unroll, hint_engines=(PE,)) as iv:
                    for u in range(unroll):
                        src_h, dst_h = h_bf[u % 2], h_bf[(u + 1) % 2]
                        dst_hf = hf[(u + 1) % 2]
                        gx = gxp.tile([P, RT * BL], bf16, tag="gx")
                        nc.sync.dma_start(
                            gx[:].rearrange("p (r b) -> p r b", b=BL),
                            gxt[:, :, ds(iv * BL + u * BL, BL)]
                            .rearrange("r p b -> p r b"),
                        )
                        pif = psp.tile([P, 2 * KB], f32, tag="pif")
                        pg = psp.tile([P, KB], f32, tag="pg")
                        po = psp.tile([P, KB], f32, tag="po")
                        for rt in range(RT):
                            if rt < 16:
                                tgt, cc = pif, rt
                            elif rt < 24:
                                tgt, cc = pg, rt - 16
                            else:
                                tgt, cc = po, rt - 24
                            for kk in range(KK):
                                nc.tensor.matmul(
                                    tgt[:, cc * BL:(cc + 1) * BL],
                                    w_sb2[:, kk * G4 + rt * P: kk * G4 + (rt + 1) * P],
                                    src_h[:, kk * BL:(kk + 1) * BL],
                                    start=(kk == 0), stop=(kk == KK - 1),
                                )
                        # gates = psum + Gx (bias already folded into Gx)
                        pre_if = elt.tile([P, 2 * KB], f32, tag="pre_if")
                        nc.vector.tensor_add(pre_if[:], pif[:], gx[:, 0:2 * KB])
                        sig_if = elt.tile([P, 2 * KB], f32, tag="sig_if")
                        nc.scalar.activation(sig_if[:], pre_if[:], AF.Sigmoid)
                        pre_g = elt.tile([P, KB], f32, tag="pre_g")
                        nc.vector.tensor_add(pre_g[:], pg[:], gx[:, 2 * KB:3 * KB])
                        tg = elt.tile([P, KB], f32, tag="tg")
                        nc.scalar.activation(tg[:], pre_g[:], AF.Tanh)
                        t1 = elt.tile([P, KB], f32, tag="t1")
                        nc.vector.tensor_mul(t1[:], sig_if[:, KB:2 * KB], cf[:])
                        t2 = elt.tile([P, KB], f32, tag="t2")
                        nc.vector.tensor_mul(t2[:], sig_if[:, 0:KB], tg[:])
                        nc.vector.tensor_add(cf[:], t1[:], t2[:])
                        tcn = elt.tile([P, KB], f32, tag="tcn")
                        nc.scalar.activation(tcn[:], cf[:], AF.Tanh)
                        pre_o = elt.tile([P, KB], f32, tag="pre_o")
                        nc.vector.tensor_add(pre_o[:], po[:], gx[:, 3 * KB:4 * KB])
                        so = elt.tile([P, KB], f32, tag="so")
                        nc.scalar.activation(so[:], pre_o[:], AF.Sigmoid)
                        nc.vector.tensor_mul(dst_hf[:], so[:], tcn[:])
                        nc.vector.tensor_copy(dst_h[:], dst_hf[:])
                        if l == 0:
                            nc.sync.dma_start(h0t[ds(iv + u, 1), :, :], dst_h[:])
                        else:
                            nc.sync.dma_start(outst.ap()[ds(iv + u, 1), :, :],
                                              dst_hf[:])
                # final states: with T even and unroll even, last write is hf[0]
                nc.sync.dma_start(hnt.ap()[l], hf[T % 2][:])
                nc.sync.dma_start(cnt.ap()[l], cf[:])

    nc.compile()
    return nc


_cached = {}


def _get_nc(T=T_FULL, unroll=2):
    key = (T, unroll)
    if key not in _cached:
        _cached[key] = _build(T, unroll)
    return _cached[key]


def _prep_in_maps(x, h, c, Wx, Wh, b, T):
    """Host-side sharding + layout transforms (cheap, O(bytes))."""
    x = np.asarray(x, np.float32)
    h = np.asarray(h, np.float32)
    c = np.asarray(c, np.float32)
    Wx = np.asarray(Wx, np.float32)
    Wh = np.asarray(Wh, np.float32)
    b = np.asarray(b, np.float32)

    wt = np.empty((L, 2, M, G4), dtype=BF16)
    for l in range(L):
        wt[l, 0] = Wx[l].T.astype(BF16)
        wt[l, 1] = Wh[l].T.astype(BF16)
    bt = b.reshape(L, RT, P).transpose(2, 0, 1).reshape(P, L * RT).copy()

    def state_t(a):  # (L, BL, M) -> (L, P, KB)
        return (a.transpose(0, 2, 1).reshape(L, KK, P, BL)
                .transpose(0, 2, 1, 3).reshape(L, P, KB).copy())

    in_maps = []
    for core in range(N_CORES):
        b0 = core * BL
        xs = x[b0:b0 + BL, :T]                       # (BL, T, N)
        xtc = (xs.transpose(1, 2, 0).reshape(T, KK, P, BL)
               .transpose(0, 2, 1, 3).reshape(T, P, KB).astype(BF16))
        in_maps.append({
            "xt": xtc,
            "wt": wt,
            "bt": bt.astype(np.float32),
            "h0": state_t(h[:, b0:b0 + BL]).astype(np.float32),
            "c0": state_t(c[:, b0:b0 + BL]).astype(np.float32),
        })
    return in_maps


def _unpack(results, T):
    outs = np.empty((B, T, M), np.float32)
    h_n = np.empty((L, B, M), np.float32)
    c_n = np.empty((L, B, M), np.float32)
    for core in range(N_CORES):
        b0 = core * BL
        r = results[core]
        outs[b0:b0 + BL] = (r["outst"].reshape(T, P, KK, BL)
                            .transpose(3, 0, 2, 1).reshape(BL, T, M))
        h_n[:, b0:b0 + BL] = (r["hnt"].reshape(L, P, KK, BL)
                              .transpose(0, 3, 2, 1).reshape(L, BL, M))
        c_n[:, b0:b0 + BL] = (r["cnt"].reshape(L, P, KK, BL)
                              .transpose(0, 3, 2, 1).reshape(L, BL, M))
    return outs, h_n, c_n


def kernel(x, h, c, Wx, Wh, b):
    T = x.shape[1]
    nc = _get_nc(T)
    in_maps = _prep_in_maps(x, h, c, Wx, Wh, b, T)
    res = run_bass_kernel_spmd(nc, in_maps, core_ids=list(range(N_CORES)))
    return _unpack(res.results, T)


# revision 7
# speedup vs baseline: 19.0328x; 19.0328x over previous
"""Trainium2 Bass kernel for a 2-layer LSTM (B=64, T=256, N=M=1024).

Strategy (data-parallel, per sharding hint):
 - Shard batch B=64 across 8 cores (8 per core). Weights replicated.
 - Host provides x in k-major transposed layout; device does FLOPs plus
   two cheap SBUF reformat passes (strided DVE copies) so every DMA in
   the hot loops is contiguous (>=512B runs) -- strided 16B-granule DMA
   was measured at ~6 GB/s and dominated runtime in v1.
 - Per core, per layer:
     bulk:   GxT = WxT_l @ inT over all timesteps (N=512 matmuls, near
             peak bf16), +bias, -> rt-major DRAM gxt.
     refmt:  gxt (rt-major) -> gxt_t (t-major) via contiguous DMA +
             strided SBUF DVE copies, so scan reads one contiguous
             (128,256) tile per step.
     scan:   256 sequential steps under tc.For_i. gates_h = Wh_l @ h_t^T
             as 256 weight-stationary (128x128)x(128x8) matmuls into
             3 PSUM banks (i|f, g, o); elementwise sigmoid/tanh chain in
             transposed layout; h stays resident in SBUF.
 - Layer-0 h stream (t-major) is reformatted to k-major h0k so layer-1
   bulk reads are contiguous too.
"""

import numpy as np
import ml_dtypes

import concourse.bass as bass
import concourse.bacc as bacc
import concourse.mybir as mybir
import concourse.tile as tile
from concourse.bass import ds
from concourse.bass_utils import run_bass_kernel_spmd

BF16 = ml_dtypes.bfloat16

N_CORES = 8
B, T_FULL, N_IN = 64, 256, 1024
M, L = 1024, 2
BL = B // N_CORES          # 8 batch rows per core
P = 128
KK = M // P                # 8 contraction tiles
RT = (4 * M) // P          # 32 gate row tiles
KB = KK * BL               # 64 = free width of state tiles
G4 = 4 * M                 # 4096
GW = RT * BL               # 256 = per-step gate tile width


def _build(T, unroll=2):
    """Build + compile the per-core program. Returns nc."""
    FD = min(512, T * BL)          # bulk free dim per matmul
    NCH = (T * BL) // FD           # bulk chunks
    TC = FD // BL                  # timesteps per bulk chunk (64)
    f32 = mybir.dt.float32
    bf16 = mybir.dt.bfloat16
    AF = mybir.ActivationFunctionType
    PE = mybir.EngineType.PE

    nc = bacc.Bacc("TRN2", target_bir_lowering=False, debug=False,
                   num_devices=N_CORES)

    # k-major x: xt[kk, p, t*BL+b] = x[b, t, kk*P+p]
    xt = nc.dram_tensor("xt", [KK, P, T * BL], bf16, kind="ExternalInput")
    # wt[l, 0] = Wx_l^T, wt[l, 1] = Wh_l^T; rows = contraction, cols = gate
    wt = nc.dram_tensor("wt", [L, 2, KK * P, G4], bf16, kind="ExternalInput")
    bt = nc.dram_tensor("bt", [P, L * RT], f32, kind="ExternalInput")
    h0 = nc.dram_tensor("h0", [L, P, KB], f32, kind="ExternalInput")
    c0 = nc.dram_tensor("c0", [L, P, KB], f32, kind="ExternalInput")
    outst = nc.dram_tensor("outst", [T, P, KB], f32, kind="ExternalOutput")
    hnt = nc.dram_tensor("hnt", [L, P, KB], f32, kind="ExternalOutput")
    cnt = nc.dram_tensor("cnt", [L, P, KB], f32, kind="ExternalOutput")

    with tile.TileContext(nc) as tc:
        with (
            tc.tile_pool(name="w", bufs=1) as wpool,
            tc.tile_pool(name="sb", bufs=2) as sb,
            tc.tile_pool(name="win", bufs=2) as winp,
            tc.tile_pool(name="gxp", bufs=3) as gxp,
            tc.tile_pool(name="elt", bufs=2) as elt,
            tc.tile_pool(name="st", bufs=1) as st,
            tc.tile_pool(name="ps", bufs=2, space="PSUM") as psp,
            tc.tile_pool(name="dram", bufs=1, space="DRAM") as dram,
        ):
            bt_sb = st.tile([P, L * RT], f32, tag="bt")
            nc.sync.dma_start(bt_sb[:], bt.ap())
            # t-major bf16 hidden stream of layer 0, then its k-major form
            h0t = dram.tile([P, T * KB], bf16, tag="h0t")
            h0k = dram.tile([KK, P, T * BL], bf16, tag="h0k")

            for l in range(L):
                # ---------------- bulk: GxT = WxT_l @ in^T + b ----------------
                src_k = xt.ap() if l == 0 else h0k[:]
                w_sb = wpool.tile([P, KK * G4], bf16, tag="w")
                nc.sync.dma_start(
                    w_sb[:].rearrange("p (kk g) -> p kk g", kk=KK),
                    wt.ap()[l, 0].rearrange("(kk p) g -> p kk g", p=P),
                )
                gxt = dram.tile([RT, P, T * BL], bf16, tag="gxt")
                for c in range(NCH):
                    rhs = sb.tile([P, KK, FD], bf16, tag="rhs")
                    nc.sync.dma_start(
                        rhs[:],
                        src_k[:, :, ds(c * FD, FD)].rearrange("kk p f -> p kk f"),
                    )
                    for rt in range(RT):
                        ps = psp.tile([P, FD], f32, tag="bulkps")
                        for kk in range(KK):
                            nc.tensor.matmul(
                                ps[:],
                                w_sb[:, kk * G4 + rt * P: kk * G4 + (rt + 1) * P],
                                rhs[:, kk, :],
                                start=(kk == 0), stop=(kk == KK - 1),
                            )
                        ev = sb.tile([P, FD], bf16, tag="ev")
                        nc.vector.tensor_scalar_add(
                            ev[:], ps[:], bt_sb[:, l * RT + rt: l * RT + rt + 1])
                        nc.sync.dma_start(gxt[rt, :, c * FD:(c + 1) * FD], ev[:])

                # -------- reformat: gxt (rt-major) -> gxt_t (t-major) --------
                gxt_t = dram.tile([P, T * GW], bf16, tag="gxt_t")
                for c in range(NCH):
                    win = winp.tile([P, TC * GW], bf16, tag="win")
                    for rt in range(RT):
                        g = sb.tile([P, FD], bf16, tag="g")
                        nc.sync.dma_start(g[:], gxt[rt, :, c * FD:(c + 1) * FD])
                        nc.vector.tensor_copy(
                            win[:].rearrange("p (t c) -> p t c", c=GW)
                                  [:, :, rt * BL:(rt + 1) * BL],
                            g[:].rearrange("p (t b) -> p t b", b=BL),
                        )
                    nc.sync.dma_start(
                        gxt_t[:, c * TC * GW:(c + 1) * TC * GW], win[:])

                # ---------------- scan over T steps ----------------
                w_sb2 = wpool.tile([P, KK * G4], bf16, tag="w")
                nc.sync.dma_start(
                    w_sb2[:].rearrange("p (kk g) -> p kk g", kk=KK),
                    wt.ap()[l, 1].rearrange("(kk p) g -> p kk g", p=P),
                )
                h_bf = [st.tile([P, KB], bf16, tag=f"hbf{j}l{l}",
                                name=f"hbf{j}l{l}") for j in range(2)]
                hf = [st.tile([P, KB], f32, tag=f"hf{j}l{l}",
                              name=f"hf{j}l{l}") for j in range(2)]
                cf = st.tile([P, KB], f32, tag=f"cfl{l}")
                nc.sync.dma_start(hf[0][:], h0.ap()[l])
                nc.sync.dma_start(cf[:], c0.ap()[l])
                nc.vector.tensor_copy(h_bf[0][:], hf[0][:])

                with tc.For_i(0, T, unroll, hint_engines=(PE,)) as iv:
                    for u in range(unroll):
                        src_h, dst_h = h_bf[u % 2], h_bf[(u + 1) % 2]
                        dst_hf = hf[(u + 1) % 2]
                        gx = gxp.tile([P, GW], bf16, tag="gx")
                        nc.sync.dma_start(
                            gx[:], gxt_t[:, ds(iv * GW + u * GW, GW)])
                        pif = psp.tile([P, 2 * KB], f32, tag="pif")
                        pg = psp.tile([P, KB], f32, tag="pg")
                        po = psp.tile([P, KB], f32, tag="po")
                        for rt in range(RT):
                            if rt < 16:
                                tgt, cc = pif, rt
                            elif rt < 24:
                                tgt, cc = pg, rt - 16
                            else:
                                tgt, cc = po, rt - 24
                            for kk in range(KK):
                                nc.tensor.matmul(
                                    tgt[:, cc * BL:(cc + 1) * BL],
                                    w_sb2[:, kk * G4 + rt * P: kk * G4 + (rt + 1) * P],
                                    src_h[:, kk * BL:(kk + 1) * BL],
                                    start=(kk == 0), stop=(kk == KK - 1),
                                )
                        # gates = psum + Gx (bias already folded into Gx)
                        pre_if = elt.tile([P, 2 * KB], f32, tag="pre_if")
                        nc.vector.tensor_add(pre_if[:], pif[:], gx[:, 0:2 * KB])
                        sig_if = elt.tile([P, 2 * KB], f32, tag="sig_if")
                        nc.scalar.activation(sig_if[:], pre_if[:], AF.Sigmoid)
                        pre_g = elt.tile([P, KB], f32, tag="pre_g")
                        nc.vector.tensor_add(pre_g[:], pg[:], gx[:, 2 * KB:3 * KB])
                        tg = elt.tile([P, KB], f32, tag="tg")
                        nc.scalar.activation(tg[:], pre_g[:], AF.Tanh)
                        t1 = elt.tile([P, KB], f32, tag="t1")
                        nc.vector.tensor_mul(t1[:], sig_if[:, KB:2 * KB], cf[:])
                        t2 = elt.tile([P, KB], f32, tag="t2")
                        nc.vector.tensor_mul(t2[:], sig_if[:, 0:KB], tg[:])
                        nc.vector.tensor_add(cf[:], t1[:], t2[:])
                        tcn = elt.tile([P, KB], f32, tag="tcn")
                        nc.scalar.activation(tcn[:], cf[:], AF.Tanh)
                        pre_o = elt.tile([P, KB], f32, tag="pre_o")
                        nc.vector.tensor_add(pre_o[:], po[:], gx[:, 3 * KB:4 * KB])
                        so = elt.tile([P, KB], f32, tag="so")
                        nc.scalar.activation(so[:], pre_o[:], AF.Sigmoid)
                        nc.vector.tensor_mul(dst_hf[:], so[:], tcn[:])
                        nc.vector.tensor_copy(dst_h[:], dst_hf[:])
                        if l == 0:
                            nc.sync.dma_start(
                                h0t[:, ds(iv * KB + u * KB, KB)], dst_h[:])
                        else:
                            nc.sync.dma_start(outst.ap()[ds(iv + u, 1), :, :],
                                              dst_hf[:])
                # final states: with T even and unroll even, last write is hf[0]
                nc.sync.dma_start(hnt.ap()[l], hf[T % 2][:])
                nc.sync.dma_start(cnt.ap()[l], cf[:])

                # ---- reformat h0t (t-major) -> h0k (k-major) for L1 bulk ----
                if l == 0:
                    for c in range(NCH):
                        hx = sb.tile([P, TC * KB], bf16, tag="hx")
                        nc.sync.dma_start(
                            hx[:], h0t[:, c * TC * KB:(c + 1) * TC * KB])
                        for kk in range(KK):
                            hk = sb.tile([P, FD], bf16, tag="hk")
                            nc.vector.tensor_copy(
                                hk[:].rearrange("p (t b) -> p t b", b=BL),
                                hx[:].rearrange("p (t c) -> p t c", c=KB)
                                     [:, :, kk * BL:(kk + 1) * BL],
                            )
                            nc.sync.dma_start(
                                h0k[kk, :, c * FD:(c + 1) * FD], hk[:])

    nc.compile()
    return nc


_cached = {}


def _get_nc(T=T_FULL, unroll=2):
    key = (T, unroll)
    if key not in _cached:
        _cached[key] = _build(T, unroll)
    return _cached[key]


def _prep_in_maps(x, h, c, Wx, Wh, b, T):
    """Host-side sharding + layout transforms (cheap, O(bytes))."""
    x = np.asarray(x, np.float32)
    h = np.asarray(h, np.float32)
    c = np.asarray(c, np.float32)
    Wx = np.asarray(Wx, np.float32)
    Wh = np.asarray(Wh, np.float32)
    b = np.asarray(b, np.float32)

    wt = np.empty((L, 2, M, G4), dtype=BF16)
    for l in range(L):
        wt[l, 0] = Wx[l].T.astype(BF16)
        wt[l, 1] = Wh[l].T.astype(BF16)
    bt = b.reshape(L, RT, P).transpose(2, 0, 1).reshape(P, L * RT).copy()

    def state_t(a):  # (L, BL, M) -> (L, P, KB)
        return (a.transpose(0, 2, 1).reshape(L, KK, P, BL)
                .transpose(0, 2, 1, 3).reshape(L, P, KB).copy())

    in_maps = []
    for core in range(N_CORES):
        b0 = core * BL
        xs = x[b0:b0 + BL, :T]                       # (BL, T, N)
        # k-major: xt[kk, p, t*BL+b] = x[b, t, kk*P+p]
        xtc = (xs.reshape(BL, T, KK, P).transpose(2, 3, 1, 0)
               .reshape(KK, P, T * BL).astype(BF16))
        in_maps.append({
            "xt": xtc,
            "wt": wt,
            "bt": bt.astype(np.float32),
            "h0": state_t(h[:, b0:b0 + BL]).astype(np.float32),
            "c0": state_t(c[:, b0:b0 + BL]).astype(np.float32),
        })
    return in_maps


def _unpack(results, T):
    outs = np.empty((B, T, M), np.float32)
    h_n = np.empty((L, B, M), np.float32)
    c_n = np.empty((L, B, M), np.float32)
    for core in range(N_CORES):
        b0 = core * BL
        r = results[core]
        outs[b0:b0 + BL] = (r["outst"].reshape(T, P, KK, BL)
                            .transpose(3, 0, 2, 1).reshape(BL, T, M))
        h_n[:, b0:b0 + BL] = (r["hnt"].reshape(L, P, KK, BL)
                              .transpose(0, 3, 2, 1).reshape(L, BL, M))
        c_n[:, b0:b0 + BL] = (r["cnt"].reshape(L, P, KK, BL)
                              .transpose(0, 3, 2, 1).reshape(L, BL, M))
    return outs, h_n, c_n


def kernel(x, h, c, Wx, Wh, b):
    T = x.shape[1]
    nc = _get_nc(T)
    in_maps = _prep_in_maps(x, h, c, Wx, Wh, b, T)
    res = run_bass_kernel_spmd(nc, in_maps, core_ids=list(range(N_CORES)))
    return _unpack(res.results, T)
